# revision 10
# baseline (speedup 1.0000x reference)
"""Multi-head latent attention kernel for Trainium2, 8 NeuronCores.

Problem (hardcoded shapes):
  hidden_states [2, 2048, 4096] f32, attention_mask [1,1,2048,2048] f32,
  Wq [4096,4096], Wk/Wv [4096,1024], Wo [4096,4096].
  4 query heads x 1024 head_dim, 1 kv head, interleaved RoPE, softmax, o-proj.

Sharding: core c = (batch b=c//4, quarter r=c%4), all within-batch groups of 4.
  - k^T / v computed from the core's sequence quarter (hsq input) and
    AllGathered (one combined collective) within the 4-core batch group.
  - Attention is sharded over QUERY positions: each core handles global
    256-row i-blocks (r, 7-r) -- a causally balanced pairing -- for ALL 4
    heads, so the output projection is fully local and there is no second
    collective. The SPMD program uses the union block structure over the 4
    cores; per-core mask tiles make each core's softmax exact.
  - Scores are computed in S^T = k q^T orientation (partitions = key index)
    so exp(S^T) feeds the PV matmul directly; the softmax denominator comes
    from a ones-vector matmul and is applied via an outer-product broadcast
    of 1/l on PV evacuation.

v2 scheduling (from baseline trace analysis):
  - Two DMA queues: sync streams weights (wk, wq0-prefetch, wv, wq1-3, wo,
    out), gpsimd carries activations/tables/kv stores/collective/kT/vT/mask.
  - Wq head 0 (24 of 32 chunks) + hqc2 + cos/sin tables prefetched to SBUF
    before the AllGather launches, so Q-head-0 compute runs DMA-free under
    the collective (the AG was observed to throttle concurrent DMA to
    ~27 GB/s and stall the PE cold).
  - hqc/hqc2 split into [128,512] tiles for fast pipeline start.
  - PSUM evacuations split between Scalar and Vector engines.
  - Mask tiles loaded once and reused across heads; phase-C PSUM
    double-buffered (bufs=8).
All matmul operands bf16 (1 cycle/row on the PE vs 4 for f32), f32 PSUM
accumulation. RoPE is a host-side deinterleave permutation of Wq/Wk columns
plus 6 elementwise ops per (even,odd) chunk pair against cos/sin tables.
The mask is handled generically: each (local block, j-chunk) is classified
on host as skip / clean / mixed; mixed blocks add a (1/SCALE)-prescaled
per-core mask tile before the exp, so causal, zero, and arbitrary additive
masks are all supported (causal skips ~40% of attention compute).
"""

import numpy as np
import ml_dtypes

from concourse import bass, mybir, tile, bacc
from concourse import bass_utils

BF16 = mybir.dt.bfloat16
F32 = mybir.dt.float32

B, S, H = 2, 2048, 4096
NH, D = 4, 1024  # query heads, head dim
PD = D // 2  # rope pair count (512)
SCALE = D ** -0.5
NCORES = 8
GROUPS = [[0, 1, 2, 3], [4, 5, 6, 7]]

KC = H // 128  # 32 contraction chunks over hidden
DC = D // 128  # 8 d-chunks of head dim
JC = S // 128  # 16 j-chunks (key) of 128
Q = S // 4  # 512, per-core kv sequence quarter
NW0 = 22  # wq head-0 chunks kept resident in SBUF (rest streamed pre-AG)

# results of the traced+profiled run (filled by kernel() when trace=True)
LAST_RESULTS = None


def _build(cats, n_mixed):
    """Build the SPMD bass program.

    cats: dict (lb, jc) -> "skip" | "clean" | int (index into packed mask
    tiles); lb in {0,1} is the local 256-row i-block, union over cores.
    """
    nc = bacc.Bacc("TRN2", target_bir_lowering=False, debug=False,
                   num_devices=NCORES)

    hsq_d = nc.dram_tensor("hsq", [H, Q], BF16, kind="ExternalInput")
    hsq2_d = nc.dram_tensor("hsq2", [H, 512], BF16, kind="ExternalInput")
    wq_d = nc.dram_tensor("wq", [H, H], BF16, kind="ExternalInput")
    wk_d = nc.dram_tensor("wk", [H, D], BF16, kind="ExternalInput")
    wv_d = nc.dram_tensor("wv", [H, D], BF16, kind="ExternalInput")
    wo_d = nc.dram_tensor("wo", [H, H], BF16, kind="ExternalInput")
    cosq_d = nc.dram_tensor("cosq", [PD, Q], BF16, kind="ExternalInput")
    sinq_d = nc.dram_tensor("sinq", [PD, Q], BF16, kind="ExternalInput")
    cosq2_d = nc.dram_tensor("cosq2", [PD, 512], BF16, kind="ExternalInput")
    sinq2_d = nc.dram_tensor("sinq2", [PD, 512], BF16, kind="ExternalInput")
    nmask = max(n_mixed, 1)
    maskp_d = nc.dram_tensor("maskp", [nmask * 128, 256], F32,
                             kind="ExternalInput")
    out_d = nc.dram_tensor("out", [512, H], F32, kind="ExternalOutput")

    # combined k+v collective bounce: rows [0:1024] = kT slice [1024, Q],
    # rows [1024:2048] = v slice [Q, 1024] flattened row-major to [1024, 512]
    kv_in = nc.dram_tensor("kv_in", [2048, 512], BF16, kind="Internal")
    kv_out = nc.dram_tensor("kv_out", [4 * 2048, 512], BF16, kind="Internal")

    PAIRS = [(0, 4), (1, 5), (2, 6), (3, 7)]

    with tile.TileContext(nc) as tc:
        with tc.tile_pool(name="pers", bufs=1) as pers:
            ones_col = pers.tile([128, 1], BF16, name="ones_col", tag="ones_col")
            nc.vector.memset(ones_col[:], 1.0)
            ones_row = pers.tile([1, 128], F32, name="ones_row", tag="ones_row")
            nc.vector.memset(ones_row[:], 1.0)
            # q^T for all 4 heads, local i columns: 32 chunks [128 d, 512 i]
            qT = [pers.tile([128, 512], BF16, name=f"qt{i}", tag=f"qt{i}")
                  for i in range(4 * DC)]
            # persistent cos/sin for Q rope (reused by all 4 heads)
            cq2 = [pers.tile([128, 512], BF16, name=f"cq2{i}", tag=f"cq2{i}")
                   for i in range(4)]
            sq2 = [pers.tile([128, 512], BF16, name=f"sq2{i}", tag=f"sq2{i}")
                   for i in range(4)]

            with tc.tile_pool(name="pq", bufs=1) as pq:
                # hqc2: Q-pass moving operand, prefetched on gpsimd queue
                hqc2 = [pq.tile([128, 512], BF16, name=f"hq2_{i}",
                                tag=f"hq2_{i}") for i in range(KC)]

                with tc.tile_pool(name="pw", bufs=1) as pw:
                    # wq head-0 resident chunks
                    wq0 = [pw.tile([128, D], BF16, name=f"wq0_{k}",
                                   tag=f"wq0_{k}") for k in range(NW0)]

                    with tc.tile_pool(name="pa", bufs=1) as pa:
                        hqc = [pa.tile([128, Q], BF16, name=f"hq_{i}",
                                       tag=f"hq_{i}") for i in range(KC)]
                        # gpsimd queue: activations first (K needs hqc[0]
                        # immediately), then K-rope tables.
                        for i in range(KC):
                            nc.gpsimd.dma_start(
                                hqc[i][:], hsq_d[128 * i:128 * (i + 1), :])
                        ck = []
                        sk = []
                        for pi in range(4):
                            c_t = pa.tile([128, Q], BF16, name="ckt",
                                          tag=f"ckt{pi}")
                            s_t = pa.tile([128, Q], BF16, name="skt",
                                          tag=f"skt{pi}")
                            nc.gpsimd.dma_start(
                                c_t[:], cosq_d[128 * pi:128 * (pi + 1), :])
                            nc.gpsimd.dma_start(
                                s_t[:], sinq_d[128 * pi:128 * (pi + 1), :])
                            ck.append(c_t)
                            sk.append(s_t)

                        def rope_pair(ps_e, ps_o, c_t, s_t, out_e, out_o, n,
                                      pool, sbufs=4):
                            """Stage psum pair to bf16 (ACT + DVE split),
                            apply rope, write outputs."""
                            st_e = pool.tile([128, n], BF16, name="stg",
                                             tag="stg", bufs=sbufs)
                            st_o = pool.tile([128, n], BF16, name="stg",
                                             tag="stg", bufs=sbufs)
                            nc.scalar.activation(
                                st_e[:], ps_e[:],
                                mybir.ActivationFunctionType.Copy)
                            nc.vector.tensor_copy(st_o[:], ps_o[:])
                            t1 = pool.tile([128, n], BF16, name="rtmp",
                                           tag="rtmp", bufs=sbufs)
                            t2 = pool.tile([128, n], BF16, name="rtmp",
                                           tag="rtmp", bufs=sbufs)
                            nc.vector.tensor_mul(t1[:], st_e[:], c_t)
                            nc.vector.tensor_mul(t2[:], st_o[:], s_t)
                            nc.vector.tensor_sub(out_e, t1[:], t2[:])
                            t3 = pool.tile([128, n], BF16, name="rtmp",
                                           tag="rtmp", bufs=sbufs)
                            t4 = pool.tile([128, n], BF16, name="rtmp",
                                           tag="rtmp", bufs=sbufs)
                            nc.vector.tensor_mul(t3[:], st_o[:], c_t)
                            nc.vector.tensor_mul(t4[:], st_e[:], s_t)
                            nc.vector.tensor_add(out_o, t3[:], t4[:])

                        with tc.tile_pool(name="paps", bufs=8,
                                          space="PSUM") as paps:
                            # --- K pass: kc-outer over 8 psum banks ---
                            with nc.named_scope("Kpass"):
                                kps = [paps.tile([128, Q], F32, name="mmps",
                                                 tag="mmps")
                                       for _ in range(DC)]
                                for kc in range(KC):
                                    wkt = pa.tile([128, D], BF16, name="wk",
                                                  tag="wk", bufs=3)
                                    nc.sync.dma_start(
                                        wkt[:],
                                        wk_d[128 * kc:128 * (kc + 1), :])
                                    # interleave wq head-0 prefetch chunks
                                    if kc < 12:
                                        nc.sync.dma_start(
                                            wq0[kc][:],
                                            wq_d[128 * kc:128 * (kc + 1),
                                                 0:D])
                                    for dc in range(DC):
                                        nc.tensor.matmul(
                                            kps[dc][:],
                                            wkt[:, 128 * dc:128 * (dc + 1)],
                                            hqc[kc][:],
                                            start=(kc == 0),
                                            stop=(kc == KC - 1))
                                for pi, (de, do) in enumerate(PAIRS):
                                    ke = pa.tile([128, Q], BF16, name="kout",
                                                 tag="kout", bufs=3)
                                    ko = pa.tile([128, Q], BF16, name="kout",
                                                 tag="kout", bufs=3)
                                    rope_pair(kps[de], kps[do], ck[pi][:],
                                              sk[pi][:], ke[:], ko[:], Q, pa,
                                              sbufs=3)
                                    nc.gpsimd.dma_start(
                                        kv_in[128 * de:128 * (de + 1), :],
                                        ke[:])
                                    nc.gpsimd.dma_start(
                                        kv_in[128 * do:128 * (do + 1), :],
                                        ko[:])

                            # hqc2 prefetch on gpsimd (lands during V pass)
                            for i in range(KC):
                                nc.gpsimd.dma_start(
                                    hqc2[i][:],
                                    hsq2_d[128 * i:128 * (i + 1), :])
                            for pi in range(4):
                                nc.gpsimd.dma_start(
                                    cq2[pi][:],
                                    cosq2_d[128 * pi:128 * (pi + 1), :])
                                nc.gpsimd.dma_start(
                                    sq2[pi][:],
                                    sinq2_d[128 * pi:128 * (pi + 1), :])

                            # --- V pass: kc-outer over 8 psum banks ---
                            with nc.named_scope("Vpass"):
                                vps = [paps.tile([128, 512], F32, name="mmps",
                                                 tag="mmps")
                                       for _ in range(8)]
                                for kc in range(KC):
                                    wvt = pa.tile([128, D], BF16, name="wv",
                                                  tag="wv", bufs=3)
                                    nc.sync.dma_start(
                                        wvt[:],
                                        wv_d[128 * kc:128 * (kc + 1), :])
                                    if 12 <= kc < NW0:
                                        nc.sync.dma_start(
                                            wq0[kc][:],
                                            wq_d[128 * kc:128 * (kc + 1),
                                                 0:D])
                                    for sc in range(4):
                                        for dvb in range(2):
                                            nc.tensor.matmul(
                                                vps[sc * 2 + dvb][:],
                                                hqc[kc][:,
                                                        128 * sc:128 * (sc + 1)],
                                                wvt[:,
                                                    512 * dvb:512 * (dvb + 1)],
                                                start=(kc == 0),
                                                stop=(kc == KC - 1))
                                # wq head-0 tail chunks: issue pre-AG so the
                                # transfers land before the collective starts
                                wq0tail = []
                                for kc in range(NW0, KC):
                                    t_ = pq.tile([128, D], BF16, name="wqs",
                                                 tag="wqs", bufs=8)
                                    nc.sync.dma_start(
                                        t_[:],
                                        wq_d[128 * kc:128 * (kc + 1), 0:D])
                                    wq0tail.append(t_)
                                for sc in range(4):
                                    for dvb in range(2):
                                        vt = pa.tile([128, 512], BF16,
                                                     name="vout", tag="vout",
                                                     bufs=3)
                                        if dvb == 0:
                                            nc.scalar.activation(
                                                vt[:], vps[sc * 2 + dvb][:],
                                                mybir.ActivationFunctionType.Copy)
                                        else:
                                            nc.vector.tensor_copy(
                                                vt[:], vps[sc * 2 + dvb][:])
                                        dst = kv_in[1024 + 256 * sc:
                                                    1024 + 256 * (sc + 1), :]
                                        dst = dst.rearrange(
                                            "(p c) f -> p c f", c=2)[:, dvb, :]
                                        nc.gpsimd.dma_start(dst, vt[:])

                            # --- combined kv AllGather (overlaps q pass) ---
                            nc.gpsimd.collective_compute(
                                "AllGather", mybir.AluOpType.bypass,
                                replica_groups=GROUPS,
                                ins=[kv_in.ap().opt()],
                                outs=[kv_out.ap().opt()])

                            # --- Q pass head 0: resident weights ---
                            with nc.named_scope("Qpass0"):
                                qps = [paps.tile([128, 512], F32, name="mmps",
                                                 tag="mmps")
                                       for _ in range(DC)]
                                for kc in range(KC):
                                    wqt = (wq0[kc] if kc < NW0
                                           else wq0tail[kc - NW0])
                                    for dc in range(DC):
                                        nc.tensor.matmul(
                                            qps[dc][:],
                                            wqt[:, 128 * dc:128 * (dc + 1)],
                                            hqc2[kc][:],
                                            start=(kc == 0),
                                            stop=(kc == KC - 1))
                                for pi, (de, do) in enumerate(PAIRS):
                                    rope_pair(qps[de], qps[do], cq2[pi][:],
                                              sq2[pi][:], qT[de][:],
                                              qT[do][:], 512, pq)


                    # pa closed (hqc freed). pw still open until here --
                    # close after head 0 (wq0 consumed above).

                    # --- Q pass heads 1-3: streamed weights ---
                    with nc.named_scope("Qpass"):
                        with tc.tile_pool(name="paps2", bufs=8,
                                          space="PSUM") as paps2:
                            for hp in range(1, 4):
                                qps = [paps2.tile([128, 512], F32,
                                                  name="mmps", tag="mmps")
                                       for _ in range(DC)]
                                for kc in range(KC):
                                    wqt = pq.tile([128, D], BF16, name="wqs",
                                                  tag="wqs", bufs=8)
                                    nc.sync.dma_start(
                                        wqt[:],
                                        wq_d[128 * kc:128 * (kc + 1),
                                             D * hp:D * (hp + 1)])
                                    for dc in range(DC):
                                        nc.tensor.matmul(
                                            qps[dc][:],
                                            wqt[:, 128 * dc:128 * (dc + 1)],
                                            hqc2[kc][:],
                                            start=(kc == 0),
                                            stop=(kc == KC - 1))
                                for pi, (de, do) in enumerate(PAIRS):
                                    rope_pair(qps[de], qps[do], cq2[pi][:],
                                              sq2[pi][:],
                                              qT[DC * hp + de][:],
                                              qT[DC * hp + do][:], 512, pq)

            # ========== phase B: attention (all heads, local i-blocks) =====
            with tc.tile_pool(name="pb", bufs=2) as pb:
                kT = [pb.tile([128, S], BF16, name=f"kt{i}", tag=f"kt{i}",
                              bufs=1) for i in range(DC)]
                for dc in range(DC):
                    for r in range(4):
                        nc.gpsimd.dma_start(
                            kT[dc][:, Q * r:Q * (r + 1)],
                            kv_out[2048 * r + 128 * dc:
                                   2048 * r + 128 * (dc + 1), :])
                vT = [pb.tile([128, D], BF16, name=f"vt{i}", tag=f"vt{i}",
                              bufs=1) for i in range(JC)]
                for jc in range(JC):
                    base = 2048 * (jc // 4) + 1024 + 256 * (jc % 4)
                    nc.gpsimd.dma_start(
                        vT[jc][:],
                        kv_out[base:base + 256, :].rearrange(
                            "(p c) f -> p (c f)", c=2))
                # mask tiles: load once, reuse across heads
                mtiles = {}
                for i in range(n_mixed):
                    mt = pb.tile([128, 256], F32, name=f"mask{i}",
                                 tag=f"mask{i}", bufs=1)
                    nc.gpsimd.dma_start(
                        mt[:], maskp_d[128 * i:128 * (i + 1), :])
                    mtiles[i] = mt
                # attention output, transposed: 32 chunks [128 hdv, 512 i]
                attnT = [pb.tile([128, 512], BF16, name=f"att{i}",
                                 tag=f"att{i}", bufs=1) for i in range(KC)]

                with nc.named_scope("attn"):
                    pbps_cm = tc.tile_pool(name="pbps", bufs=2, space="PSUM")
                    pbps = pbps_cm.__enter__()
                    for lb in range(2):
                        lsl = slice(256 * lb, 256 * (lb + 1))
                        live = [jc for jc in range(JC)
                                if cats[(lb, jc)] != "skip"]
                        for h in range(NH):
                            pT = {}
                            for jc in live:
                                sps = pbps.tile([128, 256], F32, name="sps",
                                                tag="sps", bufs=2)
                                for dc in range(DC):
                                    nc.tensor.matmul(
                                        sps[:],
                                        kT[dc][:, 128 * jc:128 * (jc + 1)],
                                        qT[DC * h + dc][:, lsl],
                                        start=(dc == 0), stop=(dc == DC - 1))
                                cat = cats[(lb, jc)]
                                if isinstance(cat, int):
                                    nc.vector.tensor_add(
                                        sps[:], sps[:], mtiles[cat][:])
                                pt = pb.tile([128, 256], BF16, name=f"pt{jc}",
                                             tag=f"pt{jc}", bufs=2)
                                nc.scalar.activation(
                                    pt[:], sps[:],
                                    mybir.ActivationFunctionType.Exp,
                                    scale=SCALE)
                                pT[jc] = pt
                            l_ps = pbps.tile([1, 256], F32, name="lps",
                                             tag="lps", bufs=1)
                            for n, jc in enumerate(live):
                                nc.tensor.matmul(l_ps[:], ones_col[:],
                                                 pT[jc][:],
                                                 start=(n == 0),
                                                 stop=(n == len(live) - 1))
                            r_sb = pb.tile([1, 256], F32, name="rsb",
                                           tag="rsb", bufs=2)
                            nc.vector.reciprocal(r_sb[:], l_ps[:])
                            r_ps = pbps.tile([128, 256], F32, name="rps",
                                             tag="rps", bufs=1)
                            nc.tensor.matmul(r_ps[:], ones_row[:], r_sb[:],
                                             start=True, stop=True)
                            rbc = pb.tile([128, 256], F32, name="rbc",
                                          tag="rbc", bufs=2)
                            nc.scalar.activation(
                                rbc[:], r_ps[:],
                                mybir.ActivationFunctionType.Copy)
                            for dc2 in range(DC):
                                pvps = pbps.tile([128, 256], F32, name="pvps",
                                                 tag="pvps", bufs=2)
                                for n, jc in enumerate(live):
                                    nc.tensor.matmul(
                                        pvps[:],
                                        vT[jc][:, 128 * dc2:128 * (dc2 + 1)],
                                        pT[jc][:], start=(n == 0),
                                        stop=(n == len(live) - 1))
                                nc.vector.tensor_mul(
                                    attnT[DC * h + dc2][:, lsl], pvps[:],
                                    rbc[:])

                    pbps_cm.__exit__(None, None, None)

                # ============ phase C: local output projection ============
                with nc.named_scope("oproj"):
                    with (
                        tc.tile_pool(name="pc", bufs=2) as pc,
                        tc.tile_pool(name="pcps", bufs=8, space="PSUM") as pcps,
                    ):
                        for eb in range(8):
                            ops = [pcps.tile([128, 512], F32, name="ops",
                                             tag="ops") for _ in range(4)]
                            for kc in range(KC):
                                wot = pc.tile([128, 512], BF16, name="wot",
                                              tag="wot", bufs=8)
                                nc.sync.dma_start(
                                    wot[:], wo_d[128 * kc:128 * (kc + 1),
                                                 512 * eb:512 * (eb + 1)])
                                for ic in range(4):
                                    nc.tensor.matmul(
                                        ops[ic][:],
                                        attnT[kc][:, 128 * ic:128 * (ic + 1)],
                                        wot[:], start=(kc == 0),
                                        stop=(kc == KC - 1))
                            for ic in range(4):
                                ot = pc.tile([128, 512], F32, name="otile",
                                             tag="otile", bufs=4)
                                if ic % 2 == 0:
                                    nc.vector.tensor_copy(ot[:], ops[ic][:])
                                else:
                                    nc.scalar.activation(
                                        ot[:], ops[ic][:],
                                        mybir.ActivationFunctionType.Copy)
                                nc.sync.dma_start(
                                    out_d[128 * ic:128 * (ic + 1),
                                          512 * eb:512 * (eb + 1)], ot[:])

    nc.compile()
    return nc


_BUILD_CACHE = {}

# core r (within its batch group) handles global 256-row i-blocks (r, 7-r)
GMAP = [(r, 7 - r) for r in range(4)]


def _classify_mask(mask):
    """Union-classify each (local block lb, jc) over the 4 quarter cores.

    Returns (cats, per-core packed mask tile arrays, n_mixed). The program
    structure (cats) is shared by all cores; mask tiles are per-core data.
    """
    m = np.asarray(mask).reshape(S, S)  # [i, j]
    cats = {}
    tiles = [[] for _ in range(4)]
    n = 0
    for lb in range(2):
        for jc in range(JC):
            blks = [m[256 * GMAP[r][lb]:256 * (GMAP[r][lb] + 1),
                      128 * jc:128 * (jc + 1)] for r in range(4)]
            if all(np.all(b <= -1e8) for b in blks):
                cats[(lb, jc)] = "skip"
            elif not any(b.any() for b in blks):
                cats[(lb, jc)] = "clean"
            else:
                cats[(lb, jc)] = n
                n += 1
                for r in range(4):
                    # [j, i] orientation, prescaled by 1/SCALE so the ACT's
                    # uniform SCALE reproduces reference's scores*SCALE + mask
                    tiles[r].append(
                        np.ascontiguousarray(blks[r].T) * (1.0 / SCALE))
    maskps = [
        np.concatenate(t, axis=0).astype(np.float32) if t
        else np.zeros((128, 256), np.float32) for t in tiles]
    return cats, maskps, n


def kernel(hidden_states, attention_mask, Wq, Wk, Wv, Wo, trace=False):
    global LAST_RESULTS
    bf = ml_dtypes.bfloat16

    cats, maskps, n_mixed = _classify_mask(attention_mask)
    key = tuple(sorted((k, v if isinstance(v, str) else "m")
                       for k, v in cats.items()))
    if key not in _BUILD_CACHE:
        _BUILD_CACHE[key] = _build(cats, n_mixed)
    nc = _BUILD_CACHE[key]

    # deinterleave rope pairs within each head's 1024 columns
    perm = np.concatenate([np.arange(0, D, 2), np.arange(1, D, 2)])
    cols = np.concatenate([h * D + perm for h in range(NH)])
    wq_p = np.ascontiguousarray(Wq[:, cols]).astype(bf)
    wk_p = np.ascontiguousarray(Wk[:, perm]).astype(bf)
    wv_c = np.ascontiguousarray(Wv).astype(bf)
    wo_c = np.ascontiguousarray(Wo).astype(bf)

    freqs = 1.0 / (10000.0 ** (np.arange(0, D, 2, dtype=np.float64) / D))
    ang = np.outer(np.arange(S, dtype=np.float64), freqs)  # [S, PD]
    cosT = np.ascontiguousarray(np.cos(ang).T).astype(bf)  # [PD, S]
    sinT = np.ascontiguousarray(np.sin(ang).T).astype(bf)

    hsT = [np.ascontiguousarray(hidden_states[b].T).astype(bf)
           for b in range(B)]

    in_maps = []
    for c in range(NCORES):
        b, r = c // 4, c % 4
        g0, g1 = GMAP[r]
        icols = np.r_[256 * g0:256 * (g0 + 1), 256 * g1:256 * (g1 + 1)]
        in_maps.append({
            "hsq": np.ascontiguousarray(hsT[b][:, Q * r:Q * (r + 1)]),
            "hsq2": np.ascontiguousarray(hsT[b][:, icols]),
            "wq": wq_p,
            "wk": wk_p,
            "wv": wv_c,
            "wo": wo_c,
            "cosq": np.ascontiguousarray(cosT[:, Q * r:Q * (r + 1)]),
            "sinq": np.ascontiguousarray(sinT[:, Q * r:Q * (r + 1)]),
            "cosq2": np.ascontiguousarray(cosT[:, icols]),
            "sinq2": np.ascontiguousarray(sinT[:, icols]),
            "maskp": maskps[r],
        })

    res = bass_utils.run_bass_kernel_spmd(
        nc, in_maps, core_ids=list(range(NCORES)), trace=trace)
    LAST_RESULTS = res

    out = np.empty((B, S, H), np.float32)
    for c in range(NCORES):
        b, r = c // 4, c % 4
        g0, g1 = GMAP[r]
        o = res.results[c]["out"]
        out[b, 256 * g0:256 * (g0 + 1), :] = o[0:256]
        out[b, 256 * g1:256 * (g1 + 1), :] = o[256:512]
    return out


# revision 20
# speedup vs baseline: 1.0233x; 1.0233x over previous
"""Multi-head latent attention kernel for Trainium2, 8 NeuronCores.

Problem (hardcoded shapes):
  hidden_states [2, 2048, 4096] f32, attention_mask [1,1,2048,2048] f32,
  Wq [4096,4096], Wk/Wv [4096,1024], Wo [4096,4096].
  4 query heads x 1024 head_dim, 1 kv head, interleaved RoPE, softmax, o-proj.

Sharding: core c = (batch b=c//4, quarter r=c%4), all within-batch groups of 4.
  - k^T / v computed from the core's sequence quarter (hsq input) and
    AllGathered (one combined collective) within the 4-core batch group.
  - Attention is sharded over QUERY positions: each core handles global
    256-row i-blocks (r, 7-r) -- a causally balanced pairing -- for ALL 4
    heads, so the output projection is fully local and there is no second
    collective. The SPMD program uses the union block structure over the 4
    cores; per-core mask tiles make each core's softmax exact.
  - Scores are computed in S^T = k q^T orientation (partitions = key index)
    so exp(S^T) feeds the PV matmul directly; the softmax denominator comes
    from a ones-vector matmul and is applied via an outer-product broadcast
    of 1/l on PV evacuation.

v2 scheduling (from baseline trace analysis):
  - Two DMA queues: sync streams weights (wk, wq0-prefetch, wv, wq1-3, wo,
    out), gpsimd carries activations/tables/kv stores/collective/kT/vT/mask.
  - Wq head 0 (24 of 32 chunks) + hqc2 + cos/sin tables prefetched to SBUF
    before the AllGather launches, so Q-head-0 compute runs DMA-free under
    the collective (the AG was observed to throttle concurrent DMA to
    ~27 GB/s and stall the PE cold).
  - hqc/hqc2 split into [128,512] tiles for fast pipeline start.
  - PSUM evacuations split between Scalar and Vector engines.
  - Mask tiles loaded once and reused across heads; phase-C PSUM
    double-buffered (bufs=8).
All matmul operands bf16 (1 cycle/row on the PE vs 4 for f32), f32 PSUM
accumulation. RoPE is a host-side deinterleave permutation of Wq/Wk columns
plus 6 elementwise ops per (even,odd) chunk pair against cos/sin tables.
The mask is handled generically: each (local block, j-chunk) is classified
on host as skip / clean / mixed; mixed blocks add a (1/SCALE)-prescaled
per-core mask tile before the exp, so causal, zero, and arbitrary additive
masks are all supported (causal skips ~40% of attention compute).
"""

import numpy as np
import ml_dtypes

from concourse import bass, mybir, tile, bacc
from concourse import bass_utils

BF16 = mybir.dt.bfloat16
F32 = mybir.dt.float32

B, S, H = 2, 2048, 4096
NH, D = 4, 1024  # query heads, head dim
PD = D // 2  # rope pair count (512)
SCALE = D ** -0.5
NCORES = 8
GROUPS = [[0, 1, 2, 3], [4, 5, 6, 7]]

KC = H // 128  # 32 contraction chunks over hidden
DC = D // 128  # 8 d-chunks of head dim
JC = S // 128  # 16 j-chunks (key) of 128
Q = S // 4  # 512, per-core kv sequence quarter
NW0 = 20  # wq head-0 chunks kept resident in SBUF (rest streamed pre-AG)

# results of the traced+profiled run (filled by kernel() when trace=True)
LAST_RESULTS = None


def _build(cats, n_mixed):
    """Build the SPMD bass program.

    cats: dict (lb, jc) -> "skip" | "clean" | int (index into packed mask
    tiles); lb in {0,1} is the local 256-row i-block, union over cores.
    """
    nc = bacc.Bacc("TRN2", target_bir_lowering=False, debug=False,
                   num_devices=NCORES)

    hsq_d = nc.dram_tensor("hsq", [H, Q], BF16, kind="ExternalInput")
    hsq2_d = nc.dram_tensor("hsq2", [H, 512], BF16, kind="ExternalInput")
    wq_d = nc.dram_tensor("wq", [H, H], BF16, kind="ExternalInput")
    wk_d = nc.dram_tensor("wk", [H, D], BF16, kind="ExternalInput")
    wv_d = nc.dram_tensor("wv", [H, D], BF16, kind="ExternalInput")
    wo_d = nc.dram_tensor("wo", [H, H], BF16, kind="ExternalInput")
    cosq_d = nc.dram_tensor("cosq", [PD, Q], BF16, kind="ExternalInput")
    sinq_d = nc.dram_tensor("sinq", [PD, Q], BF16, kind="ExternalInput")
    cosq2_d = nc.dram_tensor("cosq2", [PD, 512], BF16, kind="ExternalInput")
    sinq2_d = nc.dram_tensor("sinq2", [PD, 512], BF16, kind="ExternalInput")
    nmask = max(n_mixed, 1)
    maskp_d = nc.dram_tensor("maskp", [nmask * 128, 256], F32,
                             kind="ExternalInput")
    out_d = nc.dram_tensor("out", [512, H], F32, kind="ExternalOutput")

    # combined k+v collective bounce: rows [0:1024] = kT slice [1024, Q],
    # rows [1024:2048] = v slice [Q, 1024] flattened row-major to [1024, 512]
    kv_in = nc.dram_tensor("kv_in", [2048, 512], BF16, kind="Internal")
    kv_out = nc.dram_tensor("kv_out", [4 * 2048, 512], BF16, kind="Internal")

    PAIRS = [(0, 4), (1, 5), (2, 6), (3, 7)]

    # weight streams alternate between the sync and scalar DMA queues (each
    # queue sustains only ~100-130 GB/s; splitting doubles stream bandwidth)
    def wdma(kc, dst, src):
        (nc.sync if kc % 2 == 0 else nc.scalar).dma_start(dst, src)

    with tile.TileContext(nc) as tc:
        with tc.tile_pool(name="pers", bufs=1) as pers:
            ones_col = pers.tile([128, 1], BF16, name="ones_col", tag="ones_col")
            nc.vector.memset(ones_col[:], 1.0)
            ones_row = pers.tile([1, 128], F32, name="ones_row", tag="ones_row")
            nc.vector.memset(ones_row[:], 1.0)
            # q^T for all 4 heads, local i columns: 32 chunks [128 d, 512 i]
            qT = [pers.tile([128, 512], BF16, name=f"qt{i}", tag=f"qt{i}")
                  for i in range(4 * DC)]
            # persistent cos/sin for Q rope (reused by all 4 heads)
            cq2 = [pers.tile([128, 512], BF16, name=f"cq2{i}", tag=f"cq2{i}")
                   for i in range(4)]
            sq2 = [pers.tile([128, 512], BF16, name=f"sq2{i}", tag=f"sq2{i}")
                   for i in range(4)]

            with tc.tile_pool(name="pq", bufs=1) as pq:
                # hqc2: Q-pass moving operand, prefetched on gpsimd queue
                hqc2 = [pq.tile([128, 512], BF16, name=f"hq2_{i}",
                                tag=f"hq2_{i}") for i in range(KC)]

                with tc.tile_pool(name="pw", bufs=1) as pw:
                    # wq head-0 resident chunks
                    wq0 = [pw.tile([128, D], BF16, name=f"wq0_{k}",
                                   tag=f"wq0_{k}") for k in range(NW0)]

                    with tc.tile_pool(name="pa", bufs=1) as pa:
                        hqc = [pa.tile([128, Q], BF16, name=f"hq_{i}",
                                       tag=f"hq_{i}") for i in range(KC)]
                        # gpsimd queue: activations first (K needs hqc[0]
                        # immediately), then K-rope tables, then the wq
                        # head-0 prefetch (lands by V end, pre-AG).
                        for i in range(KC):
                            nc.gpsimd.dma_start(
                                hqc[i][:], hsq_d[128 * i:128 * (i + 1), :])
                        ck = []
                        sk = []
                        for pi in range(4):
                            c_t = pa.tile([128, Q], BF16, name="ckt",
                                          tag=f"ckt{pi}")
                            s_t = pa.tile([128, Q], BF16, name="skt",
                                          tag=f"skt{pi}")
                            nc.gpsimd.dma_start(
                                c_t[:], cosq_d[128 * pi:128 * (pi + 1), :])
                            nc.gpsimd.dma_start(
                                s_t[:], sinq_d[128 * pi:128 * (pi + 1), :])
                            ck.append(c_t)
                            sk.append(s_t)
                        for kc in range(NW0):
                            nc.gpsimd.dma_start(
                                wq0[kc][:],
                                wq_d[128 * kc:128 * (kc + 1), 0:D])

                        def rope_pair(ps_e, ps_o, c_t, s_t, out_e, out_o, n,
                                      pool, sbufs=4):
                            """Stage psum pair to bf16 (ACT + DVE split),
                            apply rope, write outputs."""
                            st_e = pool.tile([128, n], BF16, name="stg",
                                             tag="stg", bufs=sbufs)
                            st_o = pool.tile([128, n], BF16, name="stg",
                                             tag="stg", bufs=sbufs)
                            nc.scalar.activation(
                                st_e[:], ps_e[:],
                                mybir.ActivationFunctionType.Copy)
                            nc.vector.tensor_copy(st_o[:], ps_o[:])
                            t1 = pool.tile([128, n], BF16, name="rtmp",
                                           tag="rtmp", bufs=sbufs)
                            t2 = pool.tile([128, n], BF16, name="rtmp",
                                           tag="rtmp", bufs=sbufs)
                            nc.vector.tensor_mul(t1[:], st_e[:], c_t)
                            nc.vector.tensor_mul(t2[:], st_o[:], s_t)
                            nc.vector.tensor_sub(out_e, t1[:], t2[:])
                            t3 = pool.tile([128, n], BF16, name="rtmp",
                                           tag="rtmp", bufs=sbufs)
                            t4 = pool.tile([128, n], BF16, name="rtmp",
                                           tag="rtmp", bufs=sbufs)
                            nc.vector.tensor_mul(t3[:], st_o[:], c_t)
                            nc.vector.tensor_mul(t4[:], st_e[:], s_t)
                            nc.vector.tensor_add(out_o, t3[:], t4[:])

                        with tc.tile_pool(name="paps", bufs=8,
                                          space="PSUM") as paps:
                            # --- K pass: kc-outer over 8 psum banks ---
                            with nc.named_scope("Kpass"):
                                kps = [paps.tile([128, Q], F32, name="mmps",
                                                 tag="mmps")
                                       for _ in range(DC)]
                                for kc in range(KC):
                                    wkt = pa.tile([128, D], BF16, name="wk",
                                                  tag="wk", bufs=4)
                                    wdma(kc, wkt[:],
                                         wk_d[128 * kc:128 * (kc + 1), :])
                                    for dc in range(DC):
                                        nc.tensor.matmul(
                                            kps[dc][:],
                                            wkt[:, 128 * dc:128 * (dc + 1)],
                                            hqc[kc][:],
                                            start=(kc == 0),
                                            stop=(kc == KC - 1))
                                for pi, (de, do) in enumerate(PAIRS):
                                    ke = pa.tile([128, Q], BF16, name="kout",
                                                 tag="kout", bufs=3)
                                    ko = pa.tile([128, Q], BF16, name="kout",
                                                 tag="kout", bufs=3)
                                    rope_pair(kps[de], kps[do], ck[pi][:],
                                              sk[pi][:], ke[:], ko[:], Q, pa,
                                              sbufs=3)
                                    nc.gpsimd.dma_start(
                                        kv_in[128 * de:128 * (de + 1), :],
                                        ke[:])
                                    nc.gpsimd.dma_start(
                                        kv_in[128 * do:128 * (do + 1), :],
                                        ko[:])

                            # hqc2 prefetch on gpsimd (lands during V pass)
                            for i in range(KC):
                                nc.gpsimd.dma_start(
                                    hqc2[i][:],
                                    hsq2_d[128 * i:128 * (i + 1), :])
                            for pi in range(4):
                                nc.gpsimd.dma_start(
                                    cq2[pi][:],
                                    cosq2_d[128 * pi:128 * (pi + 1), :])
                                nc.gpsimd.dma_start(
                                    sq2[pi][:],
                                    sinq2_d[128 * pi:128 * (pi + 1), :])

                            # --- V pass: kc-outer over 8 psum banks ---
                            with nc.named_scope("Vpass"):
                                vps = [paps.tile([128, 512], F32, name="mmps",
                                                 tag="mmps")
                                       for _ in range(8)]
                                for kc in range(KC):
                                    wvt = pa.tile([128, D], BF16, name="wv",
                                                  tag="wv", bufs=4)
                                    wdma(kc, wvt[:],
                                         wv_d[128 * kc:128 * (kc + 1), :])
                                    for sc in range(4):
                                        for dvb in range(2):
                                            nc.tensor.matmul(
                                                vps[sc * 2 + dvb][:],
                                                hqc[kc][:,
                                                        128 * sc:128 * (sc + 1)],
                                                wvt[:,
                                                    512 * dvb:512 * (dvb + 1)],
                                                start=(kc == 0),
                                                stop=(kc == KC - 1))
                                # wq head-0 tail chunks: issue pre-AG so the
                                # transfers land before the collective starts
                                wq0tail = []
                                for kc in range(NW0, KC):
                                    t_ = pq.tile([128, D], BF16, name="wqs",
                                                 tag="wqs", bufs=9)
                                    wdma(kc, t_[:],
                                         wq_d[128 * kc:128 * (kc + 1), 0:D])
                                    wq0tail.append(t_)
                                for sc in range(4):
                                    for dvb in range(2):
                                        vt = pa.tile([128, 512], BF16,
                                                     name="vout", tag="vout",
                                                     bufs=3)
                                        if dvb == 0:
                                            nc.scalar.activation(
                                                vt[:], vps[sc * 2 + dvb][:],
                                                mybir.ActivationFunctionType.Copy)
                                        else:
                                            nc.vector.tensor_copy(
                                                vt[:], vps[sc * 2 + dvb][:])
                                        dst = kv_in[1024 + 256 * sc:
                                                    1024 + 256 * (sc + 1), :]
                                        dst = dst.rearrange(
                                            "(p c) f -> p c f", c=2)[:, dvb, :]
                                        nc.gpsimd.dma_start(dst, vt[:])

                            # --- combined kv AllGather (overlaps q pass) ---
                            nc.gpsimd.collective_compute(
                                "AllGather", mybir.AluOpType.bypass,
                                replica_groups=GROUPS,
                                ins=[kv_in.ap().opt()],
                                outs=[kv_out.ap().opt()])

                            # --- Q pass head 0: resident weights ---
                            with nc.named_scope("Qpass0"):
                                qps = [paps.tile([128, 512], F32, name="mmps",
                                                 tag="mmps")
                                       for _ in range(DC)]
                                for kc in range(KC):
                                    wqt = (wq0[kc] if kc < NW0
                                           else wq0tail[kc - NW0])
                                    for dc in range(DC):
                                        nc.tensor.matmul(
                                            qps[dc][:],
                                            wqt[:, 128 * dc:128 * (dc + 1)],
                                            hqc2[kc][:],
                                            start=(kc == 0),
                                            stop=(kc == KC - 1))
                                for pi, (de, do) in enumerate(PAIRS):
                                    rope_pair(qps[de], qps[do], cq2[pi][:],
                                              sq2[pi][:], qT[de][:],
                                              qT[do][:], 512, pq)


                    # pa closed (hqc freed). pw still open until here --
                    # close after head 0 (wq0 consumed above).

                    # --- Q pass heads 1-3: streamed weights ---
                    with nc.named_scope("Qpass"):
                        with tc.tile_pool(name="paps2", bufs=8,
                                          space="PSUM") as paps2:
                            for hp in range(1, 4):
                                qps = [paps2.tile([128, 512], F32,
                                                  name="mmps", tag="mmps")
                                       for _ in range(DC)]
                                for kc in range(KC):
                                    wqt = pq.tile([128, D], BF16, name="wqs",
                                                  tag="wqs", bufs=9)
                                    wdma(kc, wqt[:],
                                         wq_d[128 * kc:128 * (kc + 1),
                                              D * hp:D * (hp + 1)])
                                    for dc in range(DC):
                                        nc.tensor.matmul(
                                            qps[dc][:],
                                            wqt[:, 128 * dc:128 * (dc + 1)],
                                            hqc2[kc][:],
                                            start=(kc == 0),
                                            stop=(kc == KC - 1))
                                for pi, (de, do) in enumerate(PAIRS):
                                    rope_pair(qps[de], qps[do], cq2[pi][:],
                                              sq2[pi][:],
                                              qT[DC * hp + de][:],
                                              qT[DC * hp + do][:], 512, pq)

            # ========== phase B: attention (all heads, local i-blocks) =====
            with tc.tile_pool(name="pb", bufs=2) as pb:
                kT = [pb.tile([128, S], BF16, name=f"kt{i}", tag=f"kt{i}",
                              bufs=1) for i in range(DC)]
                for dc in range(DC):
                    for r in range(4):
                        nc.gpsimd.dma_start(
                            kT[dc][:, Q * r:Q * (r + 1)],
                            kv_out[2048 * r + 128 * dc:
                                   2048 * r + 128 * (dc + 1), :])
                vT = [pb.tile([128, D], BF16, name=f"vt{i}", tag=f"vt{i}",
                              bufs=1) for i in range(JC)]
                for jc in range(JC):
                    base = 2048 * (jc // 4) + 1024 + 256 * (jc % 4)
                    nc.gpsimd.dma_start(
                        vT[jc][:],
                        kv_out[base:base + 256, :].rearrange(
                            "(p c) f -> p (c f)", c=2))
                # mask tiles: load once, reuse across heads
                mtiles = {}
                for i in range(n_mixed):
                    mt = pb.tile([128, 256], F32, name=f"mask{i}",
                                 tag=f"mask{i}", bufs=1)
                    nc.gpsimd.dma_start(
                        mt[:], maskp_d[128 * i:128 * (i + 1), :])
                    mtiles[i] = mt
                # attention output, transposed: 32 chunks [128 hdv, 512 i]
                attnT = [pb.tile([128, 512], BF16, name=f"att{i}",
                                 tag=f"att{i}", bufs=1) for i in range(KC)]

                with nc.named_scope("attn"):
                    pbps_cm = tc.tile_pool(name="pbps", bufs=2, space="PSUM")
                    pbps = pbps_cm.__enter__()
                    for lb in range(2):
                        lsl = slice(256 * lb, 256 * (lb + 1))
                        live = [jc for jc in range(JC)
                                if cats[(lb, jc)] != "skip"]
                        for h in range(NH):
                            pT = {}
                            for jc in live:
                                sps = pbps.tile([128, 256], F32, name="sps",
                                                tag="sps", bufs=2)
                                for dc in range(DC):
                                    nc.tensor.matmul(
                                        sps[:],
                                        kT[dc][:, 128 * jc:128 * (jc + 1)],
                                        qT[DC * h + dc][:, lsl],
                                        start=(dc == 0), stop=(dc == DC - 1))
                                cat = cats[(lb, jc)]
                                if isinstance(cat, int):
                                    nc.vector.tensor_add(
                                        sps[:], sps[:], mtiles[cat][:])
                                pt = pb.tile([128, 256], BF16, name=f"pt{jc}",
                                             tag=f"pt{jc}", bufs=2)
                                nc.scalar.activation(
                                    pt[:], sps[:],
                                    mybir.ActivationFunctionType.Exp,
                                    scale=SCALE)
                                pT[jc] = pt
                            l_ps = pbps.tile([1, 256], F32, name="lps",
                                             tag="lps", bufs=1)
                            for n, jc in enumerate(live):
                                nc.tensor.matmul(l_ps[:], ones_col[:],
                                                 pT[jc][:],
                                                 start=(n == 0),
                                                 stop=(n == len(live) - 1))
                            r_sb = pb.tile([1, 256], F32, name="rsb",
                                           tag="rsb", bufs=2)
                            nc.vector.reciprocal(r_sb[:], l_ps[:])
                            r_ps = pbps.tile([128, 256], F32, name="rps",
                                             tag="rps", bufs=1)
                            nc.tensor.matmul(r_ps[:], ones_row[:], r_sb[:],
                                             start=True, stop=True)
                            rbc = pb.tile([128, 256], F32, name="rbc",
                                          tag="rbc", bufs=2)
                            nc.scalar.activation(
                                rbc[:], r_ps[:],
                                mybir.ActivationFunctionType.Copy)
                            for dc2 in range(DC):
                                pvps = pbps.tile([128, 256], F32, name="pvps",
                                                 tag="pvps", bufs=2)
                                for n, jc in enumerate(live):
                                    nc.tensor.matmul(
                                        pvps[:],
                                        vT[jc][:, 128 * dc2:128 * (dc2 + 1)],
                                        pT[jc][:], start=(n == 0),
                                        stop=(n == len(live) - 1))
                                nc.vector.tensor_mul(
                                    attnT[DC * h + dc2][:, lsl], pvps[:],
                                    rbc[:])

                    pbps_cm.__exit__(None, None, None)

                # ============ phase C: local output projection ============
                with nc.named_scope("oproj"):
                    with (
                        tc.tile_pool(name="pc", bufs=2) as pc,
                        tc.tile_pool(name="pcps", bufs=8, space="PSUM") as pcps,
                    ):
                        for eb in range(8):
                            ops = [pcps.tile([128, 512], F32, name="ops",
                                             tag="ops") for _ in range(4)]
                            for kc in range(KC):
                                wot = pc.tile([128, 512], BF16, name="wot",
                                              tag="wot", bufs=8)
                                wdma(kc, wot[:],
                                     wo_d[128 * kc:128 * (kc + 1),
                                          512 * eb:512 * (eb + 1)])
                                for ic in range(4):
                                    nc.tensor.matmul(
                                        ops[ic][:],
                                        attnT[kc][:, 128 * ic:128 * (ic + 1)],
                                        wot[:], start=(kc == 0),
                                        stop=(kc == KC - 1))
                            for ic in range(4):
                                ot = pc.tile([128, 512], F32, name="otile",
                                             tag="otile", bufs=4)
                                if ic % 2 == 0:
                                    nc.vector.tensor_copy(ot[:], ops[ic][:])
                                else:
                                    nc.scalar.activation(
                                        ot[:], ops[ic][:],
                                        mybir.ActivationFunctionType.Copy)
                                nc.sync.dma_start(
                                    out_d[128 * ic:128 * (ic + 1),
                                          512 * eb:512 * (eb + 1)], ot[:])

    nc.compile()
    return nc


_BUILD_CACHE = {}

# core r (within its batch group) handles global 256-row i-blocks (r, 7-r)
GMAP = [(r, 7 - r) for r in range(4)]


def _classify_mask(mask):
    """Union-classify each (local block lb, jc) over the 4 quarter cores.

    Returns (cats, per-core packed mask tile arrays, n_mixed). The program
    structure (cats) is shared by all cores; mask tiles are per-core data.
    """
    m = np.asarray(mask).reshape(S, S)  # [i, j]
    cats = {}
    tiles = [[] for _ in range(4)]
    n = 0
    for lb in range(2):
        for jc in range(JC):
            blks = [m[256 * GMAP[r][lb]:256 * (GMAP[r][lb] + 1),
                      128 * jc:128 * (jc + 1)] for r in range(4)]
            if all(np.all(b <= -1e8) for b in blks):
                cats[(lb, jc)] = "skip"
            elif not any(b.any() for b in blks):
                cats[(lb, jc)] = "clean"
            else:
                cats[(lb, jc)] = n
                n += 1
                for r in range(4):
                    # [j, i] orientation, prescaled by 1/SCALE so the ACT's
                    # uniform SCALE reproduces reference's scores*SCALE + mask
                    tiles[r].append(
                        np.ascontiguousarray(blks[r].T) * (1.0 / SCALE))
    maskps = [
        np.concatenate(t, axis=0).astype(np.float32) if t
        else np.zeros((128, 256), np.float32) for t in tiles]
    return cats, maskps, n


def kernel(hidden_states, attention_mask, Wq, Wk, Wv, Wo, trace=False):
    global LAST_RESULTS
    bf = ml_dtypes.bfloat16

    cats, maskps, n_mixed = _classify_mask(attention_mask)
    key = tuple(sorted((k, v if isinstance(v, str) else "m")
                       for k, v in cats.items()))
    if key not in _BUILD_CACHE:
        _BUILD_CACHE[key] = _build(cats, n_mixed)
    nc = _BUILD_CACHE[key]

    # deinterleave rope pairs within each head's 1024 columns
    perm = np.concatenate([np.arange(0, D, 2), np.arange(1, D, 2)])
    cols = np.concatenate([h * D + perm for h in range(NH)])
    wq_p = np.ascontiguousarray(Wq[:, cols]).astype(bf)
    wk_p = np.ascontiguousarray(Wk[:, perm]).astype(bf)
    wv_c = np.ascontiguousarray(Wv).astype(bf)
    wo_c = np.ascontiguousarray(Wo).astype(bf)

    freqs = 1.0 / (10000.0 ** (np.arange(0, D, 2, dtype=np.float64) / D))
    ang = np.outer(np.arange(S, dtype=np.float64), freqs)  # [S, PD]
    cosT = np.ascontiguousarray(np.cos(ang).T).astype(bf)  # [PD, S]
    sinT = np.ascontiguousarray(np.sin(ang).T).astype(bf)

    hsT = [np.ascontiguousarray(hidden_states[b].T).astype(bf)
           for b in range(B)]

    in_maps = []
    for c in range(NCORES):
        b, r = c // 4, c % 4
        g0, g1 = GMAP[r]
        icols = np.r_[256 * g0:256 * (g0 + 1), 256 * g1:256 * (g1 + 1)]
        in_maps.append({
            "hsq": np.ascontiguousarray(hsT[b][:, Q * r:Q * (r + 1)]),
            "hsq2": np.ascontiguousarray(hsT[b][:, icols]),
            "wq": wq_p,
            "wk": wk_p,
            "wv": wv_c,
            "wo": wo_c,
            "cosq": np.ascontiguousarray(cosT[:, Q * r:Q * (r + 1)]),
            "sinq": np.ascontiguousarray(sinT[:, Q * r:Q * (r + 1)]),
            "cosq2": np.ascontiguousarray(cosT[:, icols]),
            "sinq2": np.ascontiguousarray(sinT[:, icols]),
            "maskp": maskps[r],
        })

    res = bass_utils.run_bass_kernel_spmd(
        nc, in_maps, core_ids=list(range(NCORES)), trace=trace)
    LAST_RESULTS = res

    out = np.empty((B, S, H), np.float32)
    for c in range(NCORES):
        b, r = c // 4, c % 4
        g0, g1 = GMAP[r]
        o = res.results[c]["out"]
        out[b, 256 * g0:256 * (g0 + 1), :] = o[0:256]
        out[b, 256 * g1:256 * (g1 + 1), :] = o[256:512]
    return out


# revision 26
# speedup vs baseline: 1.0381x; 1.0144x over previous
"""Multi-head latent attention kernel for Trainium2, 8 NeuronCores.

Problem (hardcoded shapes):
  hidden_states [2, 2048, 4096] f32, attention_mask [1,1,2048,2048] f32,
  Wq [4096,4096], Wk/Wv [4096,1024], Wo [4096,4096].
  4 query heads x 1024 head_dim, 1 kv head, interleaved RoPE, softmax, o-proj.

Sharding: core c = (batch b=c//4, quarter r=c%4), all within-batch groups of 4.
  - k^T / v computed from the core's sequence quarter (hsq input) and
    AllGathered (one combined collective) within the 4-core batch group.
  - Attention is sharded over QUERY positions: each core handles global
    256-row i-blocks (r, 7-r) -- a causally balanced pairing -- for ALL 4
    heads, so the output projection is fully local and there is no second
    collective. The SPMD program uses the union block structure over the 4
    cores; per-core mask tiles make each core's softmax exact.
  - Scores are computed in S^T = k q^T orientation (partitions = key index)
    so exp(S^T) feeds the PV matmul directly; the softmax denominator comes
    from a ones-vector matmul and is applied via an outer-product broadcast
    of 1/l on PV evacuation.

v2 scheduling (from baseline trace analysis):
  - Two DMA queues: sync streams weights (wk, wq0-prefetch, wv, wq1-3, wo,
    out), gpsimd carries activations/tables/kv stores/collective/kT/vT/mask.
  - Wq head 0 (24 of 32 chunks) + hqc2 + cos/sin tables prefetched to SBUF
    before the AllGather launches, so Q-head-0 compute runs DMA-free under
    the collective (the AG was observed to throttle concurrent DMA to
    ~27 GB/s and stall the PE cold).
  - hqc/hqc2 split into [128,512] tiles for fast pipeline start.
  - PSUM evacuations split between Scalar and Vector engines.
  - Mask tiles loaded once and reused across heads; phase-C PSUM
    double-buffered (bufs=8).
All matmul operands bf16 (1 cycle/row on the PE vs 4 for f32), f32 PSUM
accumulation. RoPE is a host-side deinterleave permutation of Wq/Wk columns
plus 6 elementwise ops per (even,odd) chunk pair against cos/sin tables.
The mask is handled generically: each (local block, j-chunk) is classified
on host as skip / clean / mixed; mixed blocks add a (1/SCALE)-prescaled
per-core mask tile before the exp, so causal, zero, and arbitrary additive
masks are all supported (causal skips ~40% of attention compute).
"""

import numpy as np
import ml_dtypes

from concourse import bass, mybir, tile, bacc
from concourse import bass_utils

BF16 = mybir.dt.bfloat16
F32 = mybir.dt.float32

B, S, H = 2, 2048, 4096
NH, D = 4, 1024  # query heads, head dim
PD = D // 2  # rope pair count (512)
SCALE = D ** -0.5
NCORES = 8
GROUPS = [[0, 1, 2, 3], [4, 5, 6, 7]]

KC = H // 128  # 32 contraction chunks over hidden
DC = D // 128  # 8 d-chunks of head dim
JC = S // 128  # 16 j-chunks (key) of 128
Q = S // 4  # 512, per-core kv sequence quarter
NW0 = 20  # wq head-0 chunks kept resident in SBUF (rest streamed pre-AG)

# results of the traced+profiled run (filled by kernel() when trace=True)
LAST_RESULTS = None


def _build(cats, n_mixed):
    """Build the SPMD bass program.

    cats: dict (lb, jc) -> "skip" | "clean" | int (index into packed mask
    tiles); lb in {0,1} is the local 256-row i-block, union over cores.
    """
    nc = bacc.Bacc("TRN2", target_bir_lowering=False, debug=False,
                   num_devices=NCORES)

    hsq_d = nc.dram_tensor("hsq", [H, Q], BF16, kind="ExternalInput")
    hsq2_d = nc.dram_tensor("hsq2", [H, 512], BF16, kind="ExternalInput")
    wq_d = nc.dram_tensor("wq", [H, H], BF16, kind="ExternalInput")
    wk_d = nc.dram_tensor("wk", [H, D], BF16, kind="ExternalInput")
    wv_d = nc.dram_tensor("wv", [H, D], BF16, kind="ExternalInput")
    wo_d = nc.dram_tensor("wo", [H, H], BF16, kind="ExternalInput")
    cosq_d = nc.dram_tensor("cosq", [PD, Q], BF16, kind="ExternalInput")
    sinq_d = nc.dram_tensor("sinq", [PD, Q], BF16, kind="ExternalInput")
    cosq2_d = nc.dram_tensor("cosq2", [PD, 512], BF16, kind="ExternalInput")
    sinq2_d = nc.dram_tensor("sinq2", [PD, 512], BF16, kind="ExternalInput")
    nmask = max(n_mixed, 1)
    maskp_d = nc.dram_tensor("maskp", [nmask * 128, 256], F32,
                             kind="ExternalInput")
    out_d = nc.dram_tensor("out", [512, H], F32, kind="ExternalOutput")

    # combined k+v collective bounce: rows [0:1024] = kT slice [1024, Q],
    # rows [1024:2048] = v slice [Q, 1024] flattened row-major to [1024, 512]
    kv_in = nc.dram_tensor("kv_in", [2048, 512], BF16, kind="Internal")
    kv_out = nc.dram_tensor("kv_out", [4 * 2048, 512], BF16, kind="Internal")

    PAIRS = [(0, 4), (1, 5), (2, 6), (3, 7)]

    # weight streams alternate between the sync and scalar DMA queues (each
    # queue sustains only ~100-130 GB/s; splitting doubles stream bandwidth)
    def wdma(kc, dst, src):
        (nc.sync if kc % 2 == 0 else nc.scalar).dma_start(dst, src)

    with tile.TileContext(nc) as tc:
        with tc.tile_pool(name="pers", bufs=1) as pers:
            ones_col = pers.tile([128, 1], BF16, name="ones_col", tag="ones_col")
            nc.vector.memset(ones_col[:], 1.0)
            ones_row = pers.tile([1, 128], F32, name="ones_row", tag="ones_row")
            nc.vector.memset(ones_row[:], 1.0)
            # q^T for all 4 heads, local i columns: 32 chunks [128 d, 512 i]
            qT = [pers.tile([128, 512], BF16, name=f"qt{i}", tag=f"qt{i}")
                  for i in range(4 * DC)]
            # persistent cos/sin for Q rope (reused by all 4 heads)
            cq2 = [pers.tile([128, 512], BF16, name=f"cq2{i}", tag=f"cq2{i}")
                   for i in range(4)]
            sq2 = [pers.tile([128, 512], BF16, name=f"sq2{i}", tag=f"sq2{i}")
                   for i in range(4)]

            with tc.tile_pool(name="pq", bufs=1) as pq:
                # hqc2: Q-pass moving operand, prefetched on gpsimd queue
                hqc2 = [pq.tile([128, 512], BF16, name=f"hq2_{i}",
                                tag=f"hq2_{i}") for i in range(KC)]

                with tc.tile_pool(name="pw", bufs=1) as pw:
                    # wq head-0 resident chunks
                    wq0 = [pw.tile([128, D], BF16, name=f"wq0_{k}",
                                   tag=f"wq0_{k}") for k in range(NW0)]

                    with tc.tile_pool(name="pa", bufs=1) as pa:
                        hqc = [pa.tile([128, Q], BF16, name=f"hq_{i}",
                                       tag=f"hq_{i}") for i in range(KC)]
                        # gpsimd queue: activations first (K needs hqc[0]
                        # immediately), then K-rope tables, then the wq
                        # head-0 prefetch (lands by V end, pre-AG).
                        for i in range(KC):
                            nc.gpsimd.dma_start(
                                hqc[i][:], hsq_d[128 * i:128 * (i + 1), :])
                        ck = []
                        sk = []
                        for pi in range(4):
                            c_t = pa.tile([128, Q], BF16, name="ckt",
                                          tag=f"ckt{pi}")
                            s_t = pa.tile([128, Q], BF16, name="skt",
                                          tag=f"skt{pi}")
                            nc.gpsimd.dma_start(
                                c_t[:], cosq_d[128 * pi:128 * (pi + 1), :])
                            nc.gpsimd.dma_start(
                                s_t[:], sinq_d[128 * pi:128 * (pi + 1), :])
                            ck.append(c_t)
                            sk.append(s_t)

                        def rope_pair(ps_e, ps_o, c_t, s_t, out_e, out_o, n,
                                      pool, sbufs=4):
                            """Stage psum pair to bf16 (ACT + DVE split),
                            apply rope, write outputs."""
                            st_e = pool.tile([128, n], BF16, name="stg",
                                             tag="stg", bufs=sbufs)
                            st_o = pool.tile([128, n], BF16, name="stg",
                                             tag="stg", bufs=sbufs)
                            nc.scalar.activation(
                                st_e[:], ps_e[:],
                                mybir.ActivationFunctionType.Copy)
                            nc.vector.tensor_copy(st_o[:], ps_o[:])
                            t1 = pool.tile([128, n], BF16, name="rtmp",
                                           tag="rtmp", bufs=sbufs)
                            t2 = pool.tile([128, n], BF16, name="rtmp",
                                           tag="rtmp", bufs=sbufs)
                            nc.vector.tensor_mul(t1[:], st_e[:], c_t)
                            nc.vector.tensor_mul(t2[:], st_o[:], s_t)
                            nc.vector.tensor_sub(out_e, t1[:], t2[:])
                            t3 = pool.tile([128, n], BF16, name="rtmp",
                                           tag="rtmp", bufs=sbufs)
                            t4 = pool.tile([128, n], BF16, name="rtmp",
                                           tag="rtmp", bufs=sbufs)
                            nc.vector.tensor_mul(t3[:], st_o[:], c_t)
                            nc.vector.tensor_mul(t4[:], st_e[:], s_t)
                            nc.vector.tensor_add(out_o, t3[:], t4[:])

                        with tc.tile_pool(name="paps", bufs=8,
                                          space="PSUM") as paps:
                            # --- K pass: kc-outer over 8 psum banks ---
                            with nc.named_scope("Kpass"):
                                kps = [paps.tile([128, Q], F32, name="mmps",
                                                 tag="mmps")
                                       for _ in range(DC)]
                                for kc in range(KC):
                                    wkt = pa.tile([128, D], BF16, name="wk",
                                                  tag="wk", bufs=4)
                                    wdma(kc, wkt[:],
                                         wk_d[128 * kc:128 * (kc + 1), :])
                                    for dc in range(DC):
                                        nc.tensor.matmul(
                                            kps[dc][:],
                                            wkt[:, 128 * dc:128 * (dc + 1)],
                                            hqc[kc][:],
                                            start=(kc == 0),
                                            stop=(kc == KC - 1))
                                for pi, (de, do) in enumerate(PAIRS):
                                    ke = pa.tile([128, Q], BF16, name="kout",
                                                 tag="kout", bufs=3)
                                    ko = pa.tile([128, Q], BF16, name="kout",
                                                 tag="kout", bufs=3)
                                    rope_pair(kps[de], kps[do], ck[pi][:],
                                              sk[pi][:], ke[:], ko[:], Q, pa,
                                              sbufs=3)
                                    nc.gpsimd.dma_start(
                                        kv_in[128 * de:128 * (de + 1), :],
                                        ke[:])
                                    nc.gpsimd.dma_start(
                                        kv_in[128 * do:128 * (do + 1), :],
                                        ko[:])

                            # hqc2 + wq0 prefetch on gpsimd: issued after the
                            # K-pass kv stores, so the transfers land during
                            # the V pass and are complete before the AG
                            for i in range(KC):
                                nc.gpsimd.dma_start(
                                    hqc2[i][:],
                                    hsq2_d[128 * i:128 * (i + 1), :])
                            for kc in range(NW0):
                                nc.gpsimd.dma_start(
                                    wq0[kc][:],
                                    wq_d[128 * kc:128 * (kc + 1), 0:D])

                            # --- V pass: kc-outer over 8 psum banks ---
                            with nc.named_scope("Vpass"):
                                vps = [paps.tile([128, 512], F32, name="mmps",
                                                 tag="mmps")
                                       for _ in range(8)]
                                for kc in range(KC):
                                    wvt = pa.tile([128, D], BF16, name="wv",
                                                  tag="wv", bufs=4)
                                    wdma(kc, wvt[:],
                                         wv_d[128 * kc:128 * (kc + 1), :])
                                    for sc in range(4):
                                        for dvb in range(2):
                                            nc.tensor.matmul(
                                                vps[sc * 2 + dvb][:],
                                                hqc[kc][:,
                                                        128 * sc:128 * (sc + 1)],
                                                wvt[:,
                                                    512 * dvb:512 * (dvb + 1)],
                                                start=(kc == 0),
                                                stop=(kc == KC - 1))
                                # wq head-0 tail chunks: issue pre-AG so the
                                # transfers land before the collective starts
                                wq0tail = []
                                for kc in range(NW0, KC):
                                    t_ = pq.tile([128, D], BF16, name="wqs",
                                                 tag="wqs", bufs=9)
                                    wdma(kc, t_[:],
                                         wq_d[128 * kc:128 * (kc + 1), 0:D])
                                    wq0tail.append(t_)
                                for sc in range(4):
                                    for dvb in range(2):
                                        vt = pa.tile([128, 512], BF16,
                                                     name="vout", tag="vout",
                                                     bufs=3)
                                        if dvb == 0:
                                            nc.scalar.activation(
                                                vt[:], vps[sc * 2 + dvb][:],
                                                mybir.ActivationFunctionType.Copy)
                                        else:
                                            nc.vector.tensor_copy(
                                                vt[:], vps[sc * 2 + dvb][:])
                                        dst = kv_in[1024 + 256 * sc:
                                                    1024 + 256 * (sc + 1), :]
                                        dst = dst.rearrange(
                                            "(p c) f -> p c f", c=2)[:, dvb, :]
                                        nc.gpsimd.dma_start(dst, vt[:])

                            # --- combined kv AllGather (overlaps q pass) ---
                            nc.gpsimd.collective_compute(
                                "AllGather", mybir.AluOpType.bypass,
                                replica_groups=GROUPS,
                                ins=[kv_in.ap().opt()],
                                outs=[kv_out.ap().opt()])

                            # Q-rope tables (small; needed only by head-0
                            # rope ~60us after the AG starts)
                            for pi in range(4):
                                nc.gpsimd.dma_start(
                                    cq2[pi][:],
                                    cosq2_d[128 * pi:128 * (pi + 1), :])
                                nc.gpsimd.dma_start(
                                    sq2[pi][:],
                                    sinq2_d[128 * pi:128 * (pi + 1), :])

                            # --- Q pass head 0: resident weights ---
                            with nc.named_scope("Qpass0"):
                                qps = [paps.tile([128, 512], F32, name="mmps",
                                                 tag="mmps")
                                       for _ in range(DC)]
                                for kc in range(KC):
                                    wqt = (wq0[kc] if kc < NW0
                                           else wq0tail[kc - NW0])
                                    for dc in range(DC):
                                        nc.tensor.matmul(
                                            qps[dc][:],
                                            wqt[:, 128 * dc:128 * (dc + 1)],
                                            hqc2[kc][:],
                                            start=(kc == 0),
                                            stop=(kc == KC - 1))
                                for pi, (de, do) in enumerate(PAIRS):
                                    rope_pair(qps[de], qps[do], cq2[pi][:],
                                              sq2[pi][:], qT[de][:],
                                              qT[do][:], 512, pq)


                    # pa closed (hqc freed). pw still open until here --
                    # close after head 0 (wq0 consumed above).

                    # --- Q pass heads 1-3: streamed weights ---
                    with nc.named_scope("Qpass"):
                        with tc.tile_pool(name="paps2", bufs=8,
                                          space="PSUM") as paps2:
                            for hp in range(1, 4):
                                qps = [paps2.tile([128, 512], F32,
                                                  name="mmps", tag="mmps")
                                       for _ in range(DC)]
                                for kc in range(KC):
                                    wqt = pq.tile([128, D], BF16, name="wqs",
                                                  tag="wqs", bufs=9)
                                    wdma(kc, wqt[:],
                                         wq_d[128 * kc:128 * (kc + 1),
                                              D * hp:D * (hp + 1)])
                                    for dc in range(DC):
                                        nc.tensor.matmul(
                                            qps[dc][:],
                                            wqt[:, 128 * dc:128 * (dc + 1)],
                                            hqc2[kc][:],
                                            start=(kc == 0),
                                            stop=(kc == KC - 1))
                                for pi, (de, do) in enumerate(PAIRS):
                                    rope_pair(qps[de], qps[do], cq2[pi][:],
                                              sq2[pi][:],
                                              qT[DC * hp + de][:],
                                              qT[DC * hp + do][:], 512, pq)

            # ========== phase B: attention (all heads, local i-blocks) =====
            with tc.tile_pool(name="pb", bufs=2) as pb:
                # load kv/mask tiles in the order attention consumes them:
                # lb0 needs kT quarters 0-1, vT jc 0-7 and lb0 masks first
                kT = [pb.tile([128, S], BF16, name=f"kt{i}", tag=f"kt{i}",
                              bufs=1) for i in range(DC)]
                vT = [pb.tile([128, D], BF16, name=f"vt{i}", tag=f"vt{i}",
                              bufs=1) for i in range(JC)]
                mtiles = {}
                for i in range(n_mixed):
                    mtiles[i] = pb.tile([128, 256], F32, name=f"mask{i}",
                                        tag=f"mask{i}", bufs=1)

                def load_kt(r):
                    for dc in range(DC):
                        nc.gpsimd.dma_start(
                            kT[dc][:, Q * r:Q * (r + 1)],
                            kv_out[2048 * r + 128 * dc:
                                   2048 * r + 128 * (dc + 1), :])

                def load_vt(jc):
                    base = 2048 * (jc // 4) + 1024 + 256 * (jc % 4)
                    nc.gpsimd.dma_start(
                        vT[jc][:],
                        kv_out[base:base + 256, :].rearrange(
                            "(p c) f -> p (c f)", c=2))

                def load_masks(lb):
                    for jc in range(JC):
                        cat = cats[(lb, jc)]
                        if isinstance(cat, int):
                            nc.gpsimd.dma_start(
                                mtiles[cat][:],
                                maskp_d[128 * cat:128 * (cat + 1), :])

                load_kt(0)
                load_kt(1)
                for jc in range(8):
                    load_vt(jc)
                load_masks(0)
                load_kt(2)
                load_kt(3)
                for jc in range(8, 16):
                    load_vt(jc)
                load_masks(1)
                # attention output, transposed: 32 chunks [128 hdv, 512 i]
                attnT = [pb.tile([128, 512], BF16, name=f"att{i}",
                                 tag=f"att{i}", bufs=1) for i in range(KC)]

                with nc.named_scope("attn"):
                    pbps_cm = tc.tile_pool(name="pbps", bufs=2, space="PSUM")
                    pbps = pbps_cm.__enter__()
                    for lb in range(2):
                        lsl = slice(256 * lb, 256 * (lb + 1))
                        live = [jc for jc in range(JC)
                                if cats[(lb, jc)] != "skip"]
                        for h in range(NH):
                            pT = {}
                            for jc in live:
                                sps = pbps.tile([128, 256], F32, name="sps",
                                                tag="sps", bufs=2)
                                for dc in range(DC):
                                    nc.tensor.matmul(
                                        sps[:],
                                        kT[dc][:, 128 * jc:128 * (jc + 1)],
                                        qT[DC * h + dc][:, lsl],
                                        start=(dc == 0), stop=(dc == DC - 1))
                                cat = cats[(lb, jc)]
                                if isinstance(cat, int):
                                    nc.vector.tensor_add(
                                        sps[:], sps[:], mtiles[cat][:])
                                pt = pb.tile([128, 256], BF16, name=f"pt{jc}",
                                             tag=f"pt{jc}", bufs=2)
                                nc.scalar.activation(
                                    pt[:], sps[:],
                                    mybir.ActivationFunctionType.Exp,
                                    scale=SCALE)
                                pT[jc] = pt
                            l_ps = pbps.tile([1, 256], F32, name="lps",
                                             tag="lps", bufs=1)
                            for n, jc in enumerate(live):
                                nc.tensor.matmul(l_ps[:], ones_col[:],
                                                 pT[jc][:],
                                                 start=(n == 0),
                                                 stop=(n == len(live) - 1))
                            r_sb = pb.tile([1, 256], F32, name="rsb",
                                           tag="rsb", bufs=2)
                            nc.vector.reciprocal(r_sb[:], l_ps[:])
                            r_ps = pbps.tile([128, 256], F32, name="rps",
                                             tag="rps", bufs=1)
                            nc.tensor.matmul(r_ps[:], ones_row[:], r_sb[:],
                                             start=True, stop=True)
                            rbc = pb.tile([128, 256], F32, name="rbc",
                                          tag="rbc", bufs=2)
                            nc.scalar.activation(
                                rbc[:], r_ps[:],
                                mybir.ActivationFunctionType.Copy)
                            for dc2 in range(DC):
                                pvps = pbps.tile([128, 256], F32, name="pvps",
                                                 tag="pvps", bufs=2)
                                for n, jc in enumerate(live):
                                    nc.tensor.matmul(
                                        pvps[:],
                                        vT[jc][:, 128 * dc2:128 * (dc2 + 1)],
                                        pT[jc][:], start=(n == 0),
                                        stop=(n == len(live) - 1))
                                nc.vector.tensor_mul(
                                    attnT[DC * h + dc2][:, lsl], pvps[:],
                                    rbc[:])

                    pbps_cm.__exit__(None, None, None)

                # ============ phase C: local output projection ============
                with nc.named_scope("oproj"):
                    with (
                        tc.tile_pool(name="pc", bufs=2) as pc,
                        tc.tile_pool(name="pcps", bufs=8, space="PSUM") as pcps,
                    ):
                        for eb in range(8):
                            ops = [pcps.tile([128, 512], F32, name="ops",
                                             tag="ops") for _ in range(4)]
                            for kc in range(KC):
                                wot = pc.tile([128, 512], BF16, name="wot",
                                              tag="wot", bufs=8)
                                wdma(kc, wot[:],
                                     wo_d[128 * kc:128 * (kc + 1),
                                          512 * eb:512 * (eb + 1)])
                                for ic in range(4):
                                    nc.tensor.matmul(
                                        ops[ic][:],
                                        attnT[kc][:, 128 * ic:128 * (ic + 1)],
                                        wot[:], start=(kc == 0),
                                        stop=(kc == KC - 1))
                            for ic in range(4):
                                ot = pc.tile([128, 512], F32, name="otile",
                                             tag="otile", bufs=4)
                                if ic % 2 == 0:
                                    nc.vector.tensor_copy(ot[:], ops[ic][:])
                                else:
                                    nc.scalar.activation(
                                        ot[:], ops[ic][:],
                                        mybir.ActivationFunctionType.Copy)
                                nc.sync.dma_start(
                                    out_d[128 * ic:128 * (ic + 1),
                                          512 * eb:512 * (eb + 1)], ot[:])

    nc.compile()
    return nc


_BUILD_CACHE = {}

# core r (within its batch group) handles global 256-row i-blocks (r, 7-r)
GMAP = [(r, 7 - r) for r in range(4)]


def _classify_mask(mask):
    """Union-classify each (local block lb, jc) over the 4 quarter cores.

    Returns (cats, per-core packed mask tile arrays, n_mixed). The program
    structure (cats) is shared by all cores; mask tiles are per-core data.
    """
    m = np.asarray(mask).reshape(S, S)  # [i, j]
    cats = {}
    tiles = [[] for _ in range(4)]
    n = 0
    for lb in range(2):
        for jc in range(JC):
            blks = [m[256 * GMAP[r][lb]:256 * (GMAP[r][lb] + 1),
                      128 * jc:128 * (jc + 1)] for r in range(4)]
            if all(np.all(b <= -1e8) for b in blks):
                cats[(lb, jc)] = "skip"
            elif not any(b.any() for b in blks):
                cats[(lb, jc)] = "clean"
            else:
                cats[(lb, jc)] = n
                n += 1
                for r in range(4):
                    # [j, i] orientation, prescaled by 1/SCALE so the ACT's
                    # uniform SCALE reproduces reference's scores*SCALE + mask
                    tiles[r].append(
                        np.ascontiguousarray(blks[r].T) * (1.0 / SCALE))
    maskps = [
        np.concatenate(t, axis=0).astype(np.float32) if t
        else np.zeros((128, 256), np.float32) for t in tiles]
    return cats, maskps, n


def kernel(hidden_states, attention_mask, Wq, Wk, Wv, Wo, trace=False):
    global LAST_RESULTS
    bf = ml_dtypes.bfloat16

    cats, maskps, n_mixed = _classify_mask(attention_mask)
    key = tuple(sorted((k, v if isinstance(v, str) else "m")
                       for k, v in cats.items()))
    if key not in _BUILD_CACHE:
        _BUILD_CACHE[key] = _build(cats, n_mixed)
    nc = _BUILD_CACHE[key]

    # deinterleave rope pairs within each head's 1024 columns
    perm = np.concatenate([np.arange(0, D, 2), np.arange(1, D, 2)])
    cols = np.concatenate([h * D + perm for h in range(NH)])
    wq_p = np.ascontiguousarray(Wq[:, cols]).astype(bf)
    wk_p = np.ascontiguousarray(Wk[:, perm]).astype(bf)
    wv_c = np.ascontiguousarray(Wv).astype(bf)
    wo_c = np.ascontiguousarray(Wo).astype(bf)

    freqs = 1.0 / (10000.0 ** (np.arange(0, D, 2, dtype=np.float64) / D))
    ang = np.outer(np.arange(S, dtype=np.float64), freqs)  # [S, PD]
    cosT = np.ascontiguousarray(np.cos(ang).T).astype(bf)  # [PD, S]
    sinT = np.ascontiguousarray(np.sin(ang).T).astype(bf)

    hsT = [np.ascontiguousarray(hidden_states[b].T).astype(bf)
           for b in range(B)]

    in_maps = []
    for c in range(NCORES):
        b, r = c // 4, c % 4
        g0, g1 = GMAP[r]
        icols = np.r_[256 * g0:256 * (g0 + 1), 256 * g1:256 * (g1 + 1)]
        in_maps.append({
            "hsq": np.ascontiguousarray(hsT[b][:, Q * r:Q * (r + 1)]),
            "hsq2": np.ascontiguousarray(hsT[b][:, icols]),
            "wq": wq_p,
            "wk": wk_p,
            "wv": wv_c,
            "wo": wo_c,
            "cosq": np.ascontiguousarray(cosT[:, Q * r:Q * (r + 1)]),
            "sinq": np.ascontiguousarray(sinT[:, Q * r:Q * (r + 1)]),
            "cosq2": np.ascontiguousarray(cosT[:, icols]),
            "sinq2": np.ascontiguousarray(sinT[:, icols]),
            "maskp": maskps[r],
        })

    res = bass_utils.run_bass_kernel_spmd(
        nc, in_maps, core_ids=list(range(NCORES)), trace=trace)
    LAST_RESULTS = res

    out = np.empty((B, S, H), np.float32)
    for c in range(NCORES):
        b, r = c // 4, c % 4
        g0, g1 = GMAP[r]
        o = res.results[c]["out"]
        out[b, 256 * g0:256 * (g0 + 1), :] = o[0:256]
        out[b, 256 * g1:256 * (g1 + 1), :] = o[256:512]
    return out
